# revision 1
# baseline (speedup 1.0000x reference)
"""ANI-style AEV computer (radial + angular) on 8 Trainium2 NeuronCores.

Strategy
--------
Data-parallel over molecules (32/core), with host-side *indexing only*
(neighborlists / triple lists / one-hot bin matrices); every floating-point
operation of the AEV math runs on-device.

Angular part: the all-triples tensor is ~94% zeros under the Rca=3.5 cutoff,
so the host enumerates surviving triples (center i, neighbors j<k) into a
flat per-core list, sorted by (molecule-slot, center-half, center, species
-pair-bin).  The device computes, per 128-triple chunk:
  geometry (vectors, d^2, dot) -> 1/d and d via ACT ln/exp -> cos/sin of the
  angle -> cutoff poly -> f2 = exp(-eta (davg-shf)^2) -> f1 = q^zeta via
  exp(zeta*ln q) -> G = w*f2 (x) f1  [bf16, 32 features]
and bins G into (center, species-pair) segments with a PE matmul against a
one-hot segment matrix (PSUM-accumulated across a segment-group's chunks).

Radial part: dense over all (i,j) pairs, species-binned with a small
block-diagonal one-hot matmul.

Only one ACT table set is used (natural_log_exp): cutoff cosines are
evaluated as a degree-4 Chebyshev polynomial in u^2 (error ~1e-6), which
keeps the Activation engine free of table switches.
"""

import os
import sys

import numpy as np

for _p in ("/opt/trn_rl_repo", "/root/.axon_site/_ro/trn_rl_repo"):
    if os.path.isdir(_p) and _p not in sys.path:
        sys.path.insert(0, _p)

import concourse.bass as bass
import concourse.mybir as mybir
from concourse import bacc, tile
from concourse.bass_utils import run_bass_kernel_spmd

import ml_dtypes

AF = mybir.ActivationFunctionType
ALU = mybir.AluOpType
dt = mybir.dt
AP = bass.AP

# ---- hyperparameters (match reference) ----
NCORES = 8
M, A = 256, 24
MLOC = M // NCORES          # 32 molecules per core
RCR, RCA = 5.2, 3.5
ETA_R, ETA_A, ZETA = 16.0, 8.0, 32.0
SHF_R = np.linspace(0.9, 5.2, 17)[:-1].astype(np.float64)   # 16
SHF_A = np.linspace(0.9, 3.5, 5)[:-1].astype(np.float64)    # 4
SHF_Z = (np.arange(8) + 0.5) * np.pi / 8.0                   # 8
NPAIR, RSUB, ASUB = 10, 16, 32
NSEG = 120                  # segments per psum group = 12 centers x 10 bins
GSEG = 128                  # one-hot width (8 pad cols -> FWL weight loads)
NG = 2 * MLOC               # 64 groups/core (2 per molecule slot)
NBLK = 1                    # angular emission blocks
PGRP = 16                   # psum groups packed per PSUM bank tile
RGRP = MLOC // 4            # 8 radial groups of 4 molecules (96 = 4*24 rows)

_TRIU = np.zeros((4, 4), np.int64)
_s1, _s2 = np.triu_indices(4)
_TRIU[_s1, _s2] = np.arange(len(_s1))
_TRIU[_s2, _s1] = _TRIU[_s1, _s2]

# ---- degree-4 (in v=u^2) Chebyshev fit of cos(pi*u/2) on u in [0,1] ----
def _cos_poly():
    v = np.linspace(0.0, 1.0, 4001)
    tgt = np.cos(0.5 * np.pi * np.sqrt(v))
    from numpy.polynomial import chebyshev as C
    ch = C.Chebyshev.fit(v, tgt, 4, domain=[0, 1])
    pw = ch.convert(kind=np.polynomial.Polynomial)
    c = pw.coef  # c0..c4 in v
    K = c[4]
    a = c[:4] / K  # monic residual coeffs a0..a3
    err = np.abs(np.polyval(c[::-1], v) - tgt).max()
    return K, a, err

_POLY_K, _POLY_A, _POLY_ERR = _cos_poly()

# const tile column map ([128, 60] fp32)
_C_SHF2A = 0     # 4  : 2*shf_a
_C_SHFR = 4      # 16 : shf_r
_C_CZH = 20      # 8  : 0.5*cos(shf_z)
_C_SZH = 28      # 8  : 0.5*sin(shf_z)
_C_MASK = 36     # 24 : radial i==j mask*100 (valid on partitions 0..95)
_C_F2B = 60      # 1  : angular exp bias ln(2*K^4)
_C_RADB = 61     # 1  : radial exp bias ln(0.25*K^2)
_C_W = 62


def _build_consts():
    ct = np.zeros((128, _C_W), np.float32)
    ct[:, _C_SHF2A:_C_SHF2A + 4] = 2.0 * SHF_A
    ct[:, _C_SHFR:_C_SHFR + 16] = SHF_R
    ct[:, _C_CZH:_C_CZH + 8] = 0.5 * np.cos(SHF_Z)
    ct[:, _C_SZH:_C_SZH + 8] = 0.5 * np.sin(SHF_Z)
    mask = np.zeros((128, 24), np.float32)
    for mb in range(4):
        for j in range(24):
            mask[mb * 24 + j, j] = 100.0
    ct[:, _C_MASK:_C_MASK + 24] = mask
    K = _POLY_K
    ct[:, _C_F2B] = np.log(2.0) + 4.0 * np.log(abs(K))
    ct[:, _C_RADB] = np.log(0.25) + 2.0 * np.log(abs(K))
    return ct


# ============================================================
# host-side indexing prep (no float math enters the output path)
# ============================================================

def _prep(species, coordinates):
    sp = np.asarray(species)
    co = np.asarray(coordinates, np.float32)
    cod = co.astype(np.float64)
    vec = cod[:, None, :, :] - cod[:, :, None, :]       # [m, i, j, 3] = r_j - r_i
    dmat = np.sqrt(np.maximum((vec ** 2).sum(-1), 0.0))
    adj = (dmat <= RCA) & ~np.eye(A, dtype=bool)[None]

    # per-(m, i) neighbor lists and per-half triple counts
    nbrs = [[np.where(adj[m, i])[0] for i in range(A)] for m in range(M)]
    tri_mi = np.array([[len(nbrs[m][i]) * (len(nbrs[m][i]) - 1) // 2
                        for i in range(A)] for m in range(M)], np.int64)
    Th = np.stack([tri_mi[:, :12].sum(1), tri_mi[:, 12:].sum(1)], 1)  # [M, 2]

    # molecule -> (core, slot): sort by total triples, deal rank-groups of 8
    order = np.argsort(-(Th.sum(1)), kind="stable")
    slot2mol = np.empty((NCORES, MLOC), np.int64)
    for s in range(MLOC):
        for c in range(NCORES):
            slot2mol[c, s] = order[s * NCORES + c]

    # flat per-core triple list (sorted by slot/half/unit/p); no per-group
    # padding -- groups map to chunk RANGES (union over cores), and boundary
    # chunks matmul into more than one psum group.
    SYNCW = 4   # re-align cores to a chunk boundary every SYNCW groups
    tlo = np.zeros((NCORES, NG), np.int64)   # triple range per group
    thi = np.zeros((NCORES, NG), np.int64)
    posv = np.zeros(NCORES, np.int64)
    for g in range(NG):
        s, h = g // 2, g % 2
        if g % SYNCW == 0:
            posv[:] = int(np.ceil(posv.max() / 128.0)) * 128
        tlo[:, g] = posv
        posv += Th[slot2mol[:, s], h]
        thi[:, g] = posv
    nch = int(np.ceil(posv.max() / 128.0))
    # chunk span per group (uniform): union over cores
    clo = np.empty(NG, np.int64)
    chi = np.empty(NG, np.int64)
    for g in range(NG):
        clo[g] = (tlo[:, g] // 128).min()
        hi = np.maximum(thi[:, g] - 1, tlo[:, g]) // 128
        chi[g] = max(hi.max(), clo[g])
    span = (chi - clo + 1).astype(np.int64)
    mm_base = np.concatenate([[0], np.cumsum(span)])
    n_mm = int(mm_base[-1])

    pj = np.zeros((NCORES, 128, nch, 3), np.float32)
    pk = np.zeros((NCORES, 128, nch, 3), np.float32)
    ci = np.zeros((NCORES, 128, nch, 3), np.float32)
    oh = np.zeros((NCORES, 128, n_mm, GSEG), ml_dtypes.bfloat16)

    for c in range(NCORES):
        def put_pad(a, b, mref):
            if a >= b:
                return
            t_idx = np.arange(a, b)
            chs, ts = t_idx // 128, t_idx % 128
            pj[c, ts, chs] = mref + np.array([50, 0, 0], np.float32)
            pk[c, ts, chs] = mref + np.array([0, 50, 0], np.float32)
            ci[c, ts, chs] = mref
        prev_end = 0
        for s in range(MLOC):
            m = slot2mol[c, s]
            for h in range(2):
                g = 2 * s + h
                put_pad(prev_end, tlo[c, g], co[m, 0])  # sync-pad gap
                pos = tlo[c, g]
                for u in range(12):
                    i = h * 12 + u
                    nb = nbrs[m][i]
                    if len(nb) < 2:
                        continue
                    jj, kk = np.triu_indices(len(nb), 1)
                    j, k = nb[jj], nb[kk]
                    p = _TRIU[sp[m, j], sp[m, k]]
                    o = np.argsort(p, kind="stable")
                    j, k, p = j[o], k[o], p[o]
                    n = len(j)
                    t_idx = np.arange(pos, pos + n)
                    chs, ts = t_idx // 128, t_idx % 128
                    pj[c, ts, chs] = co[m, j]
                    pk[c, ts, chs] = co[m, k]
                    ci[c, ts, chs] = np.broadcast_to(co[m, i], (n, 3))
                    oh[c, ts, mm_base[g] + chs - clo[g], p * 12 + u] = 1
                    pos += n
                prev_end = pos
        put_pad(prev_end, nch * 128, co[slot2mol[c, 0], 0])

    # ---- radial inputs ----
    # rows: (molecule-in-block mb 0..3, atom j 0..23); groups of 4 slots
    rcj = np.zeros((NCORES, RGRP, 96, 3), np.float32)    # coords of atom j
    rcb = np.zeros((NCORES, RGRP, 96, 72), np.float32)   # molecule coords, (c,i)
    rsp = np.zeros((NCORES, RGRP, 96, 16), ml_dtypes.bfloat16)  # block-diag onehot
    for c in range(NCORES):
        for g in range(RGRP):
            for mb in range(4):
                m = slot2mol[c, g * 4 + mb]
                rows = slice(mb * 24, mb * 24 + 24)
                rcj[c, g, rows] = co[m]
                rcb[c, g, rows] = np.broadcast_to(
                    co[m].T.reshape(-1), (24, 72))
                rsp[c, g, np.arange(mb * 24, mb * 24 + 24),
                    mb * 4 + sp[m]] = 1

    meta = dict(nch=nch, n_mm=n_mm, clo=tuple(int(x) for x in clo),
                chi=tuple(int(x) for x in chi), slot2mol=slot2mol)
    arrays = dict(pj=pj, pk=pk, ci=ci, oh=oh, rcj=rcj, rcb=rcb, rsp=rsp)
    return meta, arrays


# ============================================================
# device program
# ============================================================

def _bb(ap, dims, off=0):
    """Build a broadcast/strided view: keep ap's partition dim, replace free
    dims with explicit [step, count] pairs (element units)."""
    return AP(ap.tensor, ap.offset + off,
              [list(ap.ap[0])] + [list(d) for d in dims])


def _build(nch, clo, chi):
    span = [chi[g] - clo[g] + 1 for g in range(NG)]
    mm_base = [0]
    for g in range(NG):
        mm_base.append(mm_base[-1] + span[g])
    n_mm = mm_base[-1]

    nc = bacc.Bacc(None, target_bir_lowering=False)
    pj_d = nc.declare_dram_parameter("pj", [128, nch, 3], dt.float32, False)
    pk_d = nc.declare_dram_parameter("pk", [128, nch, 3], dt.float32, False)
    ci_d = nc.declare_dram_parameter("ci", [128, nch, 3], dt.float32, False)
    oh_d = nc.declare_dram_parameter("oh", [128, n_mm, GSEG], dt.bfloat16,
                                     False)
    rcj_d = nc.declare_dram_parameter("rcj", [RGRP, 96, 3], dt.float32, False)
    rcb_d = nc.declare_dram_parameter("rcb", [RGRP, 96, 72], dt.float32, False)
    rsp_d = nc.declare_dram_parameter("rsp", [RGRP, 96, 16], dt.bfloat16, False)
    ct_d = nc.declare_dram_parameter("consts", [128, _C_W], dt.float32, False)
    outa_d = nc.declare_dram_parameter("outa", [GSEG, NG * 32], dt.float32,
                                       True)
    outr_d = nc.declare_dram_parameter("outr", [16, RGRP * 384], dt.float32,
                                       True)

    # block partition of the 64 groups
    gpb = NG // NBLK
    K, a = _POLY_K, _POLY_A
    # fold 2*K^4 (w = 2*fc_j*fc_k = 2*(K^2 s4j^2)(K^2 s4k^2)) into f2's exp bias
    F2BIAS = float(np.log(2.0) + 4.0 * np.log(abs(K)))
    # radial: rad = 0.25 * fc * exp(...) ; fc = (K*s4)^2
    RADBIAS = float(np.log(0.25) + 2.0 * np.log(abs(K)))

    with tile.TileContext(nc) as tc:
        with (
            tc.tile_pool(name="const", bufs=1) as cpool,
            tc.tile_pool(name="io", bufs=1) as io,
            tc.tile_pool(name="geo", bufs=1) as geo,
            tc.tile_pool(name="feat", bufs=1) as feat,
            tc.tile_pool(name="stg", bufs=1) as stg,
            tc.tile_pool(name="gp", bufs=3) as gp,
            tc.tile_pool(name="ps", bufs=4, space="PSUM") as ps,
        ):
            CT = cpool.tile([128, _C_W], dt.float32)
            nc.sync.dma_start(CT[:], ct_d[:])

            AZSTG = stg.tile([GSEG, NG * 32], dt.float32)   # angular staging
            RDSTG = stg.tile([16, RGRP * 384], dt.float32)  # radial staging

            V = nc.vector
            S = nc.scalar

            # angular inputs first: the geometry chain is the critical path
            PJL = io.tile([128, 3 * nch], dt.float32, tag="pj")
            PKL = io.tile([128, 3 * nch], dt.float32, tag="pk")
            CIL = io.tile([128, 3 * nch], dt.float32, tag="ci")
            OHL = io.tile([128, GSEG * n_mm], dt.bfloat16, tag="oh")
            nc.sync.dma_start(
                PJL[:].rearrange("p (n c) -> p n c", c=3), pj_d[:])
            nc.sync.dma_start(
                CIL[:].rearrange("p (n c) -> p n c", c=3), ci_d[:])
            nc.sync.dma_start(
                PKL[:].rearrange("p (n c) -> p n c", c=3), pk_d[:])

            def poly_fc(dist, nb, tag, rc, npart=128):
                """fc up to factor K^2: returns s4 with fc = (K*s4)^2."""
                u = geo.tile([npart, nb], dt.float32, tag=tag + "_u")
                # (d min rc) mult (1/rc)
                V.tensor_scalar(u[:], dist[:], rc, 1.0 / rc, ALU.min,
                                ALU.mult)
                v = geo.tile([npart, nb], dt.float32, tag=tag + "_v")
                V.tensor_tensor(v[:], u[:], u[:], ALU.mult)
                acc = geo.tile([npart, nb], dt.float32, tag=tag + "_acc")
                V.scalar_tensor_tensor(acc[:], v[:], float(a[3]), v[:],
                                       ALU.add, ALU.mult)
                V.scalar_tensor_tensor(acc[:], acc[:], float(a[2]), v[:],
                                       ALU.add, ALU.mult)
                V.scalar_tensor_tensor(acc[:], acc[:], float(a[1]), v[:],
                                       ALU.add, ALU.mult)
                V.tensor_scalar(acc[:], acc[:], float(a[0]), None, ALU.add)
                return acc

            # ---------------- radial (all 8 groups batched) ----------------
            f32, bf16 = dt.float32, dt.bfloat16
            CJ = io.tile([96, RGRP * 3], f32, tag="rcj")
            CB = io.tile([96, RGRP * 72], f32, tag="rcb")
            SP = io.tile([96, RGRP * 16], bf16, tag="rsp")
            nc.sync.dma_start(CJ[:].rearrange("p (g x) -> p g x", x=3),
                              rcj_d[:].rearrange("g p x -> p g x"))
            nc.sync.dma_start(CB[:].rearrange("p (g x) -> p g x", x=72),
                              rcb_d[:].rearrange("g p x -> p g x"))
            nc.sync.dma_start(SP[:].rearrange("p (g x) -> p g x", x=16),
                              rsp_d[:].rearrange("g p x -> p g x"))
            nc.sync.dma_start(
                OHL[:].rearrange("p (n s) -> p n s", s=GSEG), oh_d[:])

            rv = geo.tile([96, RGRP * 72], f32, tag="rv")
            V.tensor_tensor(
                _bb(rv[:], [[72, RGRP], [24, 3], [1, 24]]),
                _bb(CJ[:], [[3, RGRP], [1, 3], [0, 24]]),
                _bb(CB[:], [[72, RGRP], [24, 3], [1, 24]]),
                ALU.subtract)
            V.tensor_tensor(rv[:], rv[:], rv[:], ALU.mult)
            rd2 = geo.tile([96, RGRP * 24], f32, tag="rd2")
            V.tensor_tensor(rd2[:],
                            _bb(rv[:], [[72, RGRP], [1, 24]], off=0),
                            _bb(rv[:], [[72, RGRP], [1, 24]], off=24),
                            ALU.add)
            V.tensor_tensor(rd2[:], rd2[:],
                            _bb(rv[:], [[72, RGRP], [1, 24]], off=48),
                            ALU.add)
            V.tensor_tensor(rd2[:], rd2[:],
                            _bb(CT[:96, _C_MASK:], [[0, RGRP], [1, 24]]),
                            ALU.add)
            rdist = geo.tile([96, RGRP * 24], f32, tag="rdist")
            S.activation(rdist[:], rd2[:], AF.Ln)
            S.activation(rdist[:], rdist[:], AF.Exp, scale=0.5)

            rs4 = poly_fc(rdist, RGRP * 24, "rf", RCR, npart=96)
            rfc = geo.tile([96, RGRP * 24], f32, tag="rfc")
            V.tensor_tensor(rfc[:], rs4[:], rs4[:], ALU.mult)

            rt = feat.tile([96, RGRP * 384], f32, tag="rt")
            V.tensor_tensor(
                _bb(rt[:], [[384, RGRP], [16, 24], [1, 16]]),
                _bb(rdist[:], [[24, RGRP], [1, 24], [0, 16]]),
                _bb(CT[:96, _C_SHFR:], [[0, RGRP], [0, 24], [1, 16]]),
                ALU.subtract)
            rt2 = feat.tile([96, RGRP * 384], f32, tag="rt2")
            S.activation(rt2[:], rt[:], AF.Square)
            rex = feat.tile([96, RGRP * 384], f32, tag="rex")
            S.activation(rex[:], rt2[:], AF.Exp, scale=-ETA_R,
                         bias=CT[:96, _C_RADB:_C_RADB + 1])
            rad = feat.tile([96, RGRP * 384], bf16, tag="rad")
            V.tensor_tensor(
                _bb(rad[:], [[384, RGRP], [16, 24], [1, 16]]),
                _bb(rfc[:], [[24, RGRP], [1, 24], [0, 16]]),
                _bb(rex[:], [[384, RGRP], [16, 24], [1, 16]]),
                ALU.mult)

            for g in range(RGRP):
                rpt = ps.tile([16, 384], dt.float32, tag="rps")
                nc.tensor.matmul(rpt[:], SP[:, 16 * g:16 * (g + 1)],
                                 rad[:, 384 * g:384 * (g + 1)],
                                 start=True, stop=True)
                sl = slice(384 * g, 384 * (g + 1))
                S.activation(RDSTG[:, sl], rpt[:], AF.Copy)
                nc.sync.dma_start(outr_d[:, sl], RDSTG[:, sl])



            for b in range(NBLK):
                g0, g1 = b * gpb, (b + 1) * gpb
                c0 = clo[g0]
                c1 = chi[g1 - 1] + 1
                nb = c1 - c0
                f32, bf16 = dt.float32, dt.bfloat16

                PJ, PK, CI, OH = PJL, PKL, CIL, OHL

                vj = geo.tile([128, 3 * nb], f32, tag="vj")
                vk = geo.tile([128, 3 * nb], f32, tag="vk")
                V.tensor_tensor(vj[:], PJ[:], CI[:], ALU.subtract)
                V.tensor_tensor(vk[:], PK[:], CI[:], ALU.subtract)

                sq = geo.tile([128, 3 * nb], f32, tag="sq")
                d2j = geo.tile([128, nb], f32, tag="d2j")
                d2k = geo.tile([128, nb], f32, tag="d2k")
                dot = geo.tile([128, nb], f32, tag="dot")
                def cplane(t, cc):
                    return _bb(t[:], [[3, nb]], off=cc)
                V.tensor_tensor(sq[:], vj[:], vj[:], ALU.mult)
                V.tensor_tensor(d2j[:], cplane(sq, 0), cplane(sq, 1), ALU.add)
                V.tensor_tensor(d2j[:], d2j[:], cplane(sq, 2), ALU.add)
                V.tensor_tensor(sq[:], vk[:], vk[:], ALU.mult)
                V.tensor_tensor(d2k[:], cplane(sq, 0), cplane(sq, 1), ALU.add)
                V.tensor_tensor(d2k[:], d2k[:], cplane(sq, 2), ALU.add)
                V.tensor_tensor(sq[:], vj[:], vk[:], ALU.mult)
                V.tensor_tensor(dot[:], cplane(sq, 0), cplane(sq, 1), ALU.add)
                V.tensor_tensor(dot[:], dot[:], cplane(sq, 2), ALU.add)

                # d, 1/d via single ln + two exps (one ACT table set total)
                lj = geo.tile([128, nb], f32, tag="lj")
                lk = geo.tile([128, nb], f32, tag="lk")
                dj = geo.tile([128, nb], f32, tag="dj")
                dk = geo.tile([128, nb], f32, tag="dk")
                rj = geo.tile([128, nb], f32, tag="rj")
                rk = geo.tile([128, nb], f32, tag="rk")
                S.activation(lj[:], d2j[:], AF.Ln)
                S.activation(lk[:], d2k[:], AF.Ln)
                S.activation(dj[:], lj[:], AF.Exp, scale=0.5)
                S.activation(dk[:], lk[:], AF.Exp, scale=0.5)
                S.activation(rj[:], lj[:], AF.Exp, scale=-0.5)
                S.activation(rk[:], lk[:], AF.Exp, scale=-0.5)

                cos = geo.tile([128, nb], f32, tag="cos")
                V.scalar_tensor_tensor(cos[:], dot[:], 0.95, rj[:],
                                       ALU.mult, ALU.mult)
                V.tensor_tensor(cos[:], cos[:], rk[:], ALU.mult)
                s2 = geo.tile([128, nb], f32, tag="s2")
                V.tensor_tensor(s2[:], cos[:], cos[:], ALU.mult)
                V.tensor_scalar(s2[:], s2[:], -1.0, 1.0, ALU.mult, ALU.add)
                sin = geo.tile([128, nb], f32, tag="sin")
                S.activation(sin[:], s2[:], AF.Ln)
                S.activation(sin[:], sin[:], AF.Exp, scale=0.5)

                s4j = poly_fc(dj, nb, "fj", RCA)
                s4k = poly_fc(dk, nb, "fk", RCA)
                w2 = geo.tile([128, nb], f32, tag="w2")
                V.tensor_tensor(w2[:], s4j[:], s4k[:], ALU.mult)
                V.tensor_tensor(w2[:], w2[:], w2[:], ALU.mult)

                usum = geo.tile([128, nb], f32, tag="usum")
                V.tensor_tensor(usum[:], dj[:], dk[:], ALU.add)

                # f2[a] = exp(-eta/4*(u - 2shf_a)^2 + F2BIAS), layout (n, a)
                t4 = feat.tile([128, 4 * nb], f32, tag="t4")
                V.tensor_tensor(
                    _bb(t4[:], [[4, nb], [1, 4]]),
                    _bb(usum[:], [[1, nb], [0, 4]]),
                    _bb(CT[:, _C_SHF2A:], [[0, nb], [1, 4]]),
                    ALU.subtract)
                t4s = feat.tile([128, 4 * nb], f32, tag="t4s")
                S.activation(t4s[:], t4[:], AF.Square)
                f2 = feat.tile([128, 4 * nb], f32, tag="f2")
                S.activation(f2[:], t4s[:], AF.Exp, scale=-ETA_A / 4.0,
                             bias=CT[:, _C_F2B:_C_F2B + 1])
                wf2 = feat.tile([128, 4 * nb], f32, tag="wf2")
                V.tensor_tensor(
                    _bb(wf2[:], [[4, nb], [1, 4]]),
                    _bb(w2[:], [[1, nb], [0, 4]]),
                    _bb(f2[:], [[4, nb], [1, 4]]),
                    ALU.mult)

                # f1[z] = ((1 + cos(theta - shf_z))/2)^zeta, layout (n, z)
                q8 = feat.tile([128, 8 * nb], f32, tag="q8")
                t8 = feat.tile([128, 8 * nb], f32, tag="t8")
                V.tensor_tensor(
                    _bb(t8[:], [[8, nb], [1, 8]]),
                    _bb(cos[:], [[1, nb], [0, 8]]),
                    _bb(CT[:, _C_CZH:], [[0, nb], [1, 8]]),
                    ALU.mult)
                V.tensor_tensor(
                    _bb(q8[:], [[8, nb], [1, 8]]),
                    _bb(sin[:], [[1, nb], [0, 8]]),
                    _bb(CT[:, _C_SZH:], [[0, nb], [1, 8]]),
                    ALU.mult)
                V.scalar_tensor_tensor(q8[:], t8[:], 0.5, q8[:],
                                       ALU.add, ALU.add)
                S.activation(q8[:], q8[:], AF.Ln)
                f1 = feat.tile([128, 8 * nb], f32, tag="f1")
                S.activation(f1[:], q8[:], AF.Exp, scale=float(ZETA))

                # G[n, a, z] = wf2[n, a] * f1[n, z]   (bf16), one tile
                # per psum group so PE/copies/DMA trail the DVE slice-wise
                gwmax = max(chi[min(gt + PGRP, g1) - 1] - clo[gt] + 1
                            for gt in range(g0, g1, PGRP))
                for gt in range(g0, g1, PGRP):
                    gl = min(gt + PGRP, g1)
                    ca, cb = clo[gt] - c0, chi[gl - 1] + 1 - c0
                    Gt = gp.tile([128, 32 * gwmax], bf16, tag="G")
                    V.tensor_tensor(
                        _bb(Gt[:], [[32, cb - ca], [8, 4], [1, 8]]),
                        _bb(wf2[:, 4 * ca:], [[4, cb - ca], [1, 4], [0, 8]]),
                        _bb(f1[:, 8 * ca:], [[8, cb - ca], [0, 4], [1, 8]]),
                        ALU.mult)
                    pt = ps.tile([GSEG, 32 * PGRP], dt.float32, tag="ps")
                    for g in range(gt, gl):
                        gi = g - gt
                        for k in range(span[g]):
                            cc = clo[g] + k           # absolute chunk
                            nc.tensor.matmul(
                                pt[:, 32 * gi:32 * (gi + 1)],
                                OH[:, GSEG * (mm_base[g] + k):
                                      GSEG * (mm_base[g] + k + 1)],
                                Gt[:, 32 * (cc - c0 - ca):
                                      32 * (cc - c0 - ca + 1)],
                                start=(k == 0), stop=(k == span[g] - 1))
                    gbase = gt
                    sl = slice(32 * gbase, 32 * (gbase + PGRP))
                    S.activation(AZSTG[:, sl], pt[:], AF.Copy)
                    nc.sync.dma_start(outa_d[:, sl], AZSTG[:, sl])

    _patch_act_tables()
    nc.compile()
    return nc


_ACT_PATCHED = False


def _patch_act_tables():
    """Make Ln/Exp resolve only to the combined natural_log_exp set, so the
    table-load pass emits ONE load instead of thrashing between the ln-only
    and exp-only sets (1.28us per reload)."""
    global _ACT_PATCHED
    if _ACT_PATCHED:
        return
    orig = bacc.get_activation_tables

    def patched(arch):
        t = dict(orig(arch))
        out = {}
        for name, fns in t.items():
            if name != "natural_log_exp_and_others":
                fns = {f for f in fns if f not in (AF.Ln, AF.Exp)}
            out[name] = fns
        return out

    bacc.get_activation_tables = patched
    _ACT_PATCHED = True


_CACHE = {}


def kernel(species, coordinates, coefficients=None):
    species = np.asarray(species)
    coordinates = np.asarray(coordinates, np.float32)
    meta, arrays = _prep(species, coordinates)
    key = (meta["nch"], meta["clo"], meta["chi"])
    if key not in _CACHE:
        _CACHE[key] = _build(meta["nch"], list(meta["clo"]),
                             list(meta["chi"]))
    nc = _CACHE[key]

    ct = _build_consts()
    in_maps = []
    for c in range(NCORES):
        in_maps.append({
            "pj": arrays["pj"][c], "pk": arrays["pk"][c],
            "ci": arrays["ci"][c], "oh": arrays["oh"][c],
            "rcj": arrays["rcj"][c], "rcb": arrays["rcb"][c],
            "rsp": arrays["rsp"][c], "consts": ct,
        })
    res = run_bass_kernel_spmd(nc, in_maps, core_ids=list(range(NCORES)))
    out = np.empty((M, A, 384), np.float32)
    for c in range(NCORES):
        outa = np.asarray(res.results[c]["outa"])  # [128, NG*32]
        outr = np.asarray(res.results[c]["outr"])  # [16, RGRP*384]
        ang = outa.reshape(GSEG, NG, 32)[:120]
        ang = ang.reshape(10, 12, MLOC, 2, 32)          # [p, u, s, h, az]
        ang = ang.transpose(2, 3, 1, 0, 4).reshape(MLOC, A, 320)
        rad = outr.reshape(4, 4, RGRP, A, 16)           # [mb, sp, g, i, r]
        rad = rad.transpose(2, 0, 3, 1, 4).reshape(MLOC, A, 64)
        out[meta["slot2mol"][c], :, :64] = rad
        out[meta["slot2mol"][c], :, 64:] = ang
    return out



# revision 8
# speedup vs baseline: 1.0036x; 1.0036x over previous
"""ANI-style AEV computer (radial + angular) on 8 Trainium2 NeuronCores.

Strategy
--------
Data-parallel over molecules (32/core), host-side *indexing only*; all AEV
float math runs on-device.

Angular: host enumerates surviving triples (center i, neighbors j<k within
Rca) into a flat per-core list sorted by (slot, half, center, pair-bin).
Device computes geometry -> cutoffs -> f2/f1 -> G (bf16, 32 feats) per
128-triple chunk and bins G into (center, species-pair) segments with PE
matmuls against per-chunk one-hot matrices (PSUM-accumulated per group).

Radial: dense over all (i,j) pairs, 768 rows packed as 6x128 partitions,
species-binned with small block one-hot matmuls packed 4-up into a PSUM
bank via tile_position col-tiling.  The radial elementwise chain runs on
GpSimd to keep DVE free for the angular chain.

Engine budget: DVE ~ angular chain + rt, ACT ~ all exp/ln (one table set),
GpSimd ~ radial elementwise, PE ~ binning matmuls, outputs in bf16.
"""

import os
import sys

import numpy as np

for _p in ("/opt/trn_rl_repo", "/root/.axon_site/_ro/trn_rl_repo"):
    if os.path.isdir(_p) and _p not in sys.path:
        sys.path.insert(0, _p)

import concourse.bass as bass
import concourse.mybir as mybir
from concourse import bacc, tile
from concourse.bass_utils import run_bass_kernel_spmd

import ml_dtypes

AF = mybir.ActivationFunctionType
ALU = mybir.AluOpType
dt = mybir.dt
AP = bass.AP

# ---- hyperparameters (match reference) ----
NCORES = 8
M, A = 256, 24
MLOC = M // NCORES          # 32 molecules per core
RCR, RCA = 5.2, 3.5
ETA_R, ETA_A, ZETA = 16.0, 8.0, 32.0
SHF_R = np.linspace(0.9, 5.2, 17)[:-1].astype(np.float64)   # 16
SHF_A = np.linspace(0.9, 3.5, 5)[:-1].astype(np.float64)    # 4
SHF_Z = (np.arange(8) + 0.5) * np.pi / 8.0                   # 8
NPAIR, RSUB, ASUB = 10, 16, 32
NSEG = 120                  # segments per psum group = 12 centers x 10 bins
GSEG = 128                  # one-hot width
NG = 2 * MLOC               # 64 groups/core (2 per molecule slot)
PGRP = 16                   # psum groups packed per PSUM bank tile
RG = 6                      # radial groups: 768 rows = 6 x 128
RSEGW = 32                  # radial one-hot width per group

_TRIU = np.zeros((4, 4), np.int64)
_s1, _s2 = np.triu_indices(4)
_TRIU[_s1, _s2] = np.arange(len(_s1))
_TRIU[_s2, _s1] = _TRIU[_s1, _s2]

# ---- degree-4 (in v=u^2) Chebyshev fit of cos(pi*u/2) on u in [0,1] ----
def _cos_poly():
    v = np.linspace(0.0, 1.0, 4001)
    tgt = np.cos(0.5 * np.pi * np.sqrt(v))
    from numpy.polynomial import chebyshev as C
    ch = C.Chebyshev.fit(v, tgt, 4, domain=[0, 1])
    pw = ch.convert(kind=np.polynomial.Polynomial)
    c = pw.coef  # c0..c4 in v
    K = c[4]
    a = c[:4] / K  # monic residual coeffs a0..a3
    return K, a

_POLY_K, _POLY_A = _cos_poly()

# const tile column map ([128, _C_W] fp32)
_C_SHF2A = 0     # 4  : 2*shf_a
_C_SHFR = 4      # 16 : shf_r
_C_CZH = 20      # 4  : 0.5*cos(shf_z[0:4])
_C_SZH = 24      # 4  : 0.5*sin(shf_z[0:4])
_C_MASK = 28     # 144: radial self-pair mask*100, 6 groups x 24 centers
_C_F2B = 172     # 1  : angular exp bias ln(2*K^4)
_C_RADB = 173    # 1  : radial exp bias ln(0.25*K^2)
_C_HALF = 174    # 1  : 0.5 (ACT bias column)
_C_NLRCR = 175   # 1  : -ln(RCR) (ACT bias for u = d/RCR)
_C_A3 = 176      # 4  : poly coeffs a3,a2,a1,a0 (GP TT consts)
_C_W = 180


def _build_consts():
    ct = np.zeros((128, _C_W), np.float32)
    ct[:, _C_SHF2A:_C_SHF2A + 4] = 2.0 * SHF_A
    ct[:, _C_SHFR:_C_SHFR + 16] = SHF_R
    ct[:, _C_CZH:_C_CZH + 4] = 0.5 * np.cos(SHF_Z[:4])
    ct[:, _C_SZH:_C_SZH + 4] = 0.5 * np.sin(SHF_Z[:4])
    # mask[p, g*24 + i] = 100 if center i == neighbor j of global row g*128+p
    mask = np.zeros((128, RG * 24), np.float32)
    for g in range(RG):
        for p in range(128):
            j = (g * 128 + p) % 24
            mask[p, g * 24 + j] = 100.0
    ct[:, _C_MASK:_C_MASK + RG * 24] = mask
    K = _POLY_K
    ct[:, _C_F2B] = np.log(2.0) + 4.0 * np.log(abs(K))
    ct[:, _C_RADB] = np.log(0.25) + 2.0 * np.log(abs(K))
    ct[:, _C_HALF] = 0.5
    ct[:, _C_NLRCR] = -np.log(RCR)
    ct[:, _C_A3:_C_A3 + 4] = np.array(_POLY_A[::-1], np.float32)  # a3,a2,a1,a0
    return ct


# ============================================================
# host-side indexing prep (no float math enters the output path)
# ============================================================

def _prep(species, coordinates):
    sp = np.asarray(species)
    co = np.asarray(coordinates, np.float32)
    cod = co.astype(np.float64)
    vec = cod[:, None, :, :] - cod[:, :, None, :]       # [m, i, j, 3] = r_j - r_i
    dmat = np.sqrt(np.maximum((vec ** 2).sum(-1), 0.0))
    adj = (dmat <= RCA) & ~np.eye(A, dtype=bool)[None]

    nbrs = [[np.where(adj[m, i])[0] for i in range(A)] for m in range(M)]
    tri_mi = np.array([[len(nbrs[m][i]) * (len(nbrs[m][i]) - 1) // 2
                        for i in range(A)] for m in range(M)], np.int64)
    Th = np.stack([tri_mi[:, :12].sum(1), tri_mi[:, 12:].sum(1)], 1)  # [M, 2]

    # molecule -> (core, slot): sort by total triples, deal rank-groups of 8
    order = np.argsort(-(Th.sum(1)), kind="stable")
    slot2mol = np.empty((NCORES, MLOC), np.int64)
    for s in range(MLOC):
        for c in range(NCORES):
            slot2mol[c, s] = order[s * NCORES + c]

    SYNCW = 4   # re-align cores to a chunk boundary every SYNCW groups
    tlo = np.zeros((NCORES, NG), np.int64)
    thi = np.zeros((NCORES, NG), np.int64)
    posv = np.zeros(NCORES, np.int64)
    for g in range(NG):
        s, h = g // 2, g % 2
        if g % SYNCW == 0:
            posv[:] = int(np.ceil(posv.max() / 128.0)) * 128
        tlo[:, g] = posv
        posv += Th[slot2mol[:, s], h]
        thi[:, g] = posv
    nch = int(np.ceil(posv.max() / 128.0))
    clo = np.empty(NG, np.int64)
    chi = np.empty(NG, np.int64)
    for g in range(NG):
        clo[g] = (tlo[:, g] // 128).min()
        hi = np.maximum(thi[:, g] - 1, tlo[:, g]) // 128
        chi[g] = max(hi.max(), clo[g])
    span = (chi - clo + 1).astype(np.int64)
    mm_base = np.concatenate([[0], np.cumsum(span)])
    n_mm = int(mm_base[-1])

    # angular inputs: ang = [pj | pk | ci] cols, each [nch*3]
    ang = np.zeros((NCORES, 128, 3 * 3 * nch), np.float32)
    pj = ang[:, :, 0:3 * nch].reshape(NCORES, 128, nch, 3)
    pk = ang[:, :, 3 * nch:6 * nch].reshape(NCORES, 128, nch, 3)
    ci = ang[:, :, 6 * nch:9 * nch].reshape(NCORES, 128, nch, 3)
    oh = np.zeros((NCORES, 128, n_mm, GSEG), ml_dtypes.bfloat16)

    for c in range(NCORES):
        def put_pad(a, b, mref):
            if a >= b:
                return
            t_idx = np.arange(a, b)
            chs, ts = t_idx // 128, t_idx % 128
            pj[c, ts, chs] = mref + np.array([50, 0, 0], np.float32)
            pk[c, ts, chs] = mref + np.array([0, 50, 0], np.float32)
            ci[c, ts, chs] = mref
        prev_end = 0
        for s in range(MLOC):
            m = slot2mol[c, s]
            for h in range(2):
                g = 2 * s + h
                put_pad(prev_end, tlo[c, g], co[m, 0])
                pos = tlo[c, g]
                for u in range(12):
                    i = h * 12 + u
                    nb = nbrs[m][i]
                    if len(nb) < 2:
                        continue
                    jj, kk = np.triu_indices(len(nb), 1)
                    j, k = nb[jj], nb[kk]
                    p = _TRIU[sp[m, j], sp[m, k]]
                    o = np.argsort(p, kind="stable")
                    j, k, p = j[o], k[o], p[o]
                    n = len(j)
                    t_idx = np.arange(pos, pos + n)
                    chs, ts = t_idx // 128, t_idx % 128
                    pj[c, ts, chs] = co[m, j]
                    pk[c, ts, chs] = co[m, k]
                    ci[c, ts, chs] = np.broadcast_to(co[m, i], (n, 3))
                    oh[c, ts, mm_base[g] + chs - clo[g], p * 12 + u] = 1
                    pos += n
                prev_end = pos
        put_pad(prev_end, nch * 128, co[slot2mol[c, 0], 0])

    # ---- radial inputs: 768 rows = (slot, neighbor j), 6 groups of 128 ----
    # radf = [rcj (6*3) | rcb (6*72)] cols
    radf = np.zeros((NCORES, 128, RG * 3 + RG * 72), np.float32)
    rcj = radf[:, :, :RG * 3].reshape(NCORES, 128, RG, 3)
    rcb = radf[:, :, RG * 3:].reshape(NCORES, 128, RG, 72)
    rsp = np.zeros((NCORES, 128, RG * RSEGW), ml_dtypes.bfloat16)
    s0g = [(128 * g) // 24 for g in range(RG)]   # first slot in group g
    for c in range(NCORES):
        for g in range(RG):
            for p in range(128):
                r = g * 128 + p
                s, j = r // 24, r % 24
                m = slot2mol[c, s]
                rcj[c, p, g] = co[m, j]
                rcb[c, p, g] = co[m].T.reshape(-1)
                rsp[c, p, g * RSEGW + (s - s0g[g]) * 4 + sp[m, j]] = 1

    meta = dict(nch=nch, n_mm=n_mm, clo=tuple(int(x) for x in clo),
                chi=tuple(int(x) for x in chi), slot2mol=slot2mol,
                s0g=s0g)
    arrays = dict(ang=ang, oh=oh, radf=radf, rsp=rsp)
    return meta, arrays


# ============================================================
# device program
# ============================================================

def _bb(ap, dims, off=0):
    """Build a broadcast/strided view: keep ap's partition dim, replace free
    dims with explicit [step, count] pairs (element units)."""
    return AP(ap.tensor, ap.offset + off,
              [list(ap.ap[0])] + [list(d) for d in dims])


def _build(nch, clo, chi):
    span = [chi[g] - clo[g] + 1 for g in range(NG)]
    mm_base = [0]
    for g in range(NG):
        mm_base.append(mm_base[-1] + span[g])
    n_mm = mm_base[-1]
    nb = nch

    nc = bacc.Bacc(None, target_bir_lowering=False)
    ang_d = nc.declare_dram_parameter("ang", [128, 9 * nch], dt.float32, False)
    oh_d = nc.declare_dram_parameter("oh", [128, n_mm, GSEG], dt.bfloat16,
                                     False)
    radf_d = nc.declare_dram_parameter("radf", [128, RG * 75], dt.float32,
                                       False)
    rsp_d = nc.declare_dram_parameter("rsp", [128, RG * RSEGW], dt.bfloat16,
                                      False)
    ct_d = nc.declare_dram_parameter("consts", [128, _C_W], dt.float32, False)
    outa_d = nc.declare_dram_parameter("outa", [GSEG, NG * 32], dt.bfloat16,
                                       True)
    outr_d = nc.declare_dram_parameter("outr", [128, 2 * 24 * 16],
                                       dt.bfloat16, True)

    Ka, a = _POLY_K, _POLY_A
    F2BIAS = float(np.log(2.0) + 4.0 * np.log(abs(Ka)))

    f32, bf16 = dt.float32, dt.bfloat16

    with tile.TileContext(nc) as tc:
        with (
            tc.tile_pool(name="const", bufs=1) as cpool,
            tc.tile_pool(name="io", bufs=1) as io,
            tc.tile_pool(name="geo", bufs=1) as geo,
            tc.tile_pool(name="feat", bufs=1) as feat,
            tc.tile_pool(name="stg", bufs=1) as stg,
            tc.tile_pool(name="gp", bufs=3) as gp,
            tc.tile_pool(name="ps", bufs=4, space="PSUM") as ps,
            tc.tile_pool(name="psr", bufs=2, space="PSUM") as psr,
        ):
            V = nc.vector
            S = nc.scalar
            G = nc.gpsimd

            CT = cpool.tile([128, _C_W], f32)
            nc.sync.dma_start(CT[:], ct_d[:])

            # ---- input DMAs: ANG on scalar queue (first), rest on sync ----
            ANG = io.tile([128, 9 * nch], f32, tag="ang")
            S.dma_start(ANG[:], ang_d[:])
            RADF = io.tile([128, RG * 75], f32, tag="radf")
            nc.sync.dma_start(RADF[:], radf_d[:])
            RSP = io.tile([128, RG * RSEGW], bf16, tag="rsp")
            nc.sync.dma_start(RSP[:], rsp_d[:])
            OHL = io.tile([128, GSEG * n_mm], bf16, tag="oh")
            # split one-hot per PGRP block so early matmuls don't wait on all
            ohsplit = [mm_base[min(b * PGRP, NG)] for b in range(NG // PGRP + 1)]
            for b in range(NG // PGRP):
                lo, hi = ohsplit[b], ohsplit[b + 1]
                if hi > lo:
                    nc.sync.dma_start(
                        OHL[:, GSEG * lo:GSEG * hi].rearrange(
                            "p (n s) -> p n s", s=GSEG),
                        oh_d[:, lo:hi])

            AZSTG = stg.tile([GSEG, NG * 32], bf16)     # angular staging
            RDSTG = stg.tile([128, 768], bf16)          # radial staging

            # =========== radial elementwise chain (GpSimd + ACT) ===========
            CJ = RADF[:, :RG * 3]
            CB = RADF[:, RG * 3:]
            rv = geo.tile([128, RG * 72], f32, tag="rv")
            G.tensor_tensor(
                _bb(rv[:], [[72, RG], [24, 3], [1, 24]]),
                _bb(CJ, [[3, RG], [1, 3], [0, 24]]),
                _bb(CB, [[72, RG], [24, 3], [1, 24]]),
                ALU.subtract)
            rvs = geo.tile([128, RG * 72], f32, tag="rvs")
            G.tensor_tensor(rvs[:], rv[:], rv[:], ALU.mult)
            rt1 = geo.tile([128, RG * 24], f32, tag="rt1")
            G.tensor_tensor(rt1[:],
                            _bb(rvs[:], [[72, RG], [1, 24]], off=0),
                            _bb(rvs[:], [[72, RG], [1, 24]], off=24),
                            ALU.add)
            rt2m = geo.tile([128, RG * 24], f32, tag="rt2m")
            G.tensor_tensor(rt2m[:],
                            _bb(rvs[:], [[72, RG], [1, 24]], off=48),
                            CT[:, _C_MASK:_C_MASK + RG * 24],
                            ALU.add)
            rd2 = geo.tile([128, RG * 24], f32, tag="rd2")
            G.tensor_tensor(rd2[:], rt1[:], rt2m[:], ALU.add)

            rln = geo.tile([128, RG * 24], f32, tag="rln")
            S.activation(rln[:], rd2[:], AF.Ln)
            rdist = geo.tile([128, RG * 24], f32, tag="rdist")
            S.activation(rdist[:], rln[:], AF.Exp, scale=0.5)
            # u = d/RCR unclamped (spurious fc beyond RCR is killed by the
            # radial gaussian: max error ~3e-5 absolute)
            ru = geo.tile([128, RG * 24], f32, tag="ru")
            S.activation(ru[:], rln[:], AF.Exp, scale=0.5,
                         bias=CT[:, _C_NLRCR:_C_NLRCR + 1])

            # cutoff poly on GpSimd (tensor_tensor-only; Pool has no
            # tensor_scalar in the ISA)
            def ctcol(col, n):
                return _bb(CT[:, col:col + 1], [[0, n]])
            N = RG * 24
            rvv = geo.tile([128, N], f32, tag="rvv")
            G.tensor_tensor(rvv[:], ru[:], ru[:], ALU.mult)
            racc = geo.tile([128, N], f32, tag="racc")
            G.tensor_tensor(racc[:], rvv[:], ctcol(_C_A3, N), ALU.add)
            G.tensor_tensor(racc[:], racc[:], rvv[:], ALU.mult)
            G.tensor_tensor(racc[:], racc[:], ctcol(_C_A3 + 1, N), ALU.add)
            G.tensor_tensor(racc[:], racc[:], rvv[:], ALU.mult)
            G.tensor_tensor(racc[:], racc[:], ctcol(_C_A3 + 2, N), ALU.add)
            G.tensor_tensor(racc[:], racc[:], rvv[:], ALU.mult)
            G.tensor_tensor(racc[:], racc[:], ctcol(_C_A3 + 3, N), ALU.add)
            rfc = geo.tile([128, N], f32, tag="rfc")
            G.tensor_tensor(rfc[:], racc[:], racc[:], ALU.mult)

            # rt = dist - shf_r  (DVE, wide)
            rt = feat.tile([128, RG * 384], f32, tag="rt")
            V.tensor_tensor(
                _bb(rt[:], [[384, RG], [16, 24], [1, 16]]),
                _bb(rdist[:], [[24, RG], [1, 24], [0, 16]]),
                _bb(CT[:, _C_SHFR:], [[0, RG], [0, 24], [1, 16]]),
                ALU.subtract)
            rsq = feat.tile([128, RG * 384], f32, tag="rsq")
            S.activation(rsq[:], rt[:], AF.Square)
            rex = feat.tile([128, RG * 384], f32, tag="rex")
            S.activation(rex[:], rsq[:], AF.Exp, scale=-ETA_R,
                         bias=CT[:, _C_RADB:_C_RADB + 1])
            rad = feat.tile([128, RG * 384], bf16, tag="rad")
            G.tensor_tensor(
                _bb(rad[:], [[384, 4], [16, 24], [1, 16]]),
                _bb(rfc[:], [[24, 4], [1, 24], [0, 16]]),
                _bb(rex[:], [[384, 4], [16, 24], [1, 16]]),
                ALU.mult)
            V.tensor_tensor(
                _bb(rad[:], [[384, 2], [16, 24], [1, 16]], off=4 * 384),
                _bb(rfc[:], [[24, 2], [1, 24], [0, 16]], off=4 * 24),
                _bb(rex[:], [[384, 2], [16, 24], [1, 16]], off=4 * 384),
                ALU.mult)

            # radial binning matmuls: 4-up col-tiled into 2 PSUM banks
            rpt0 = psr.tile([128, 384], f32, tag="rps0")
            rpt1 = psr.tile([128, 384], f32, tag="rps1")
            for g in range(RG):
                pt = rpt0 if g < 4 else rpt1
                j = g % 4
                nc.tensor.matmul(pt[32 * j:32 * (j + 1), :],
                                 RSP[:, RSEGW * g:RSEGW * (g + 1)],
                                 rad[:, 384 * g:384 * (g + 1)],
                                 start=True, stop=True,
                                 tile_position=(0, 32 * j))
            S.activation(RDSTG[:, 0:384], rpt0[:], AF.Copy)
            S.activation(RDSTG[:64, 384:768], rpt1[:64, :], AF.Copy)
            nc.sync.dma_start(outr_d[:, 0:384], RDSTG[:, 0:384])
            nc.sync.dma_start(outr_d[:64, 384:768], RDSTG[:64, 384:768])

            # ================= angular chain (DVE + ACT) =================
            PJPK = ANG[:, 0:6 * nch]
            CI = ANG[:, 6 * nch:9 * nch]

            VJK = geo.tile([128, 6 * nb], f32, tag="vjk")
            V.tensor_tensor(VJK[:], PJPK,
                            _bb(CI, [[0, 2], [1, 3 * nb]]),
                            ALU.subtract)
            SQP = geo.tile([128, 9 * nb], f32, tag="sqp")
            V.tensor_tensor(SQP[:, 0:6 * nb], VJK[:], VJK[:], ALU.mult)
            V.tensor_tensor(SQP[:, 6 * nb:9 * nb], VJK[:, 0:3 * nb],
                            VJK[:, 3 * nb:6 * nb], ALU.mult)
            T1 = geo.tile([128, 3 * nb], f32, tag="t1")
            V.tensor_tensor(_bb(T1[:], [[nb, 3], [1, nb]]),
                            _bb(SQP[:], [[3 * nb, 3], [3, nb]], off=0),
                            _bb(SQP[:], [[3 * nb, 3], [3, nb]], off=1),
                            ALU.add)
            D2 = geo.tile([128, 3 * nb], f32, tag="d2")
            V.tensor_tensor(_bb(D2[:], [[nb, 3], [1, nb]]),
                            _bb(T1[:], [[nb, 3], [1, nb]]),
                            _bb(SQP[:], [[3 * nb, 3], [3, nb]], off=2),
                            ALU.add)

            # [lnd2j|lnd2k] -> [dj|dk], [rj|rk]
            L2 = geo.tile([128, 2 * nb], f32, tag="l2")
            S.activation(L2[:], D2[:, 0:2 * nb], AF.Ln)
            DD = geo.tile([128, 2 * nb], f32, tag="dd")
            S.activation(DD[:], L2[:], AF.Exp, scale=0.5)
            RR = geo.tile([128, 2 * nb], f32, tag="rr")
            S.activation(RR[:], L2[:], AF.Exp, scale=-0.5)

            # rt interleaves here on V (after D2); cos chain next
            rjrk = geo.tile([128, nb], f32, tag="rjrk")
            V.tensor_tensor(rjrk[:], RR[:, 0:nb], RR[:, nb:2 * nb], ALU.mult)
            cos = geo.tile([128, nb], f32, tag="cos")
            V.scalar_tensor_tensor(cos[:], D2[:, 2 * nb:3 * nb], 0.95,
                                   rjrk[:], ALU.mult, ALU.mult)
            c2 = geo.tile([128, nb], f32, tag="c2")
            V.tensor_tensor(c2[:], cos[:], cos[:], ALU.mult)
            s2 = geo.tile([128, nb], f32, tag="s2")
            V.tensor_scalar(s2[:], c2[:], -1.0, 1.0, ALU.mult, ALU.add)
            sln = geo.tile([128, nb], f32, tag="sln")
            S.activation(sln[:], s2[:], AF.Ln)
            sin = geo.tile([128, nb], f32, tag="sin")
            S.activation(sin[:], sln[:], AF.Exp, scale=0.5)

            # cutoff poly for [dj|dk] merged
            au = geo.tile([128, 2 * nb], f32, tag="au")
            V.tensor_scalar(au[:], DD[:], RCA, 1.0 / RCA, ALU.min, ALU.mult)
            av = geo.tile([128, 2 * nb], f32, tag="av")
            V.tensor_tensor(av[:], au[:], au[:], ALU.mult)
            aacc = geo.tile([128, 2 * nb], f32, tag="aacc")
            V.scalar_tensor_tensor(aacc[:], av[:], float(a[3]), av[:],
                                   ALU.add, ALU.mult)
            V.scalar_tensor_tensor(aacc[:], aacc[:], float(a[2]), av[:],
                                   ALU.add, ALU.mult)
            V.scalar_tensor_tensor(aacc[:], aacc[:], float(a[1]), av[:],
                                   ALU.add, ALU.mult)
            V.tensor_scalar(aacc[:], aacc[:], float(a[0]), None, ALU.add)
            wm = geo.tile([128, nb], f32, tag="wm")
            V.tensor_tensor(wm[:], aacc[:, 0:nb], aacc[:, nb:2 * nb],
                            ALU.mult)
            w2 = geo.tile([128, nb], f32, tag="w2")
            V.tensor_tensor(w2[:], wm[:], wm[:], ALU.mult)
            usum = geo.tile([128, nb], f32, tag="usum")
            V.tensor_tensor(usum[:], DD[:, 0:nb], DD[:, nb:2 * nb], ALU.add)

            # f2[a] side: t4 = usum - 2shf_a, layout (t, a)
            t4 = feat.tile([128, 4 * nb], f32, tag="t4")
            V.tensor_tensor(
                _bb(t4[:], [[4, nb], [1, 4]]),
                _bb(usum[:], [[1, nb], [0, 4]]),
                _bb(CT[:, _C_SHF2A:], [[0, nb], [1, 4]]),
                ALU.subtract)
            t4s = feat.tile([128, 4 * nb], f32, tag="t4s")
            S.activation(t4s[:], t4[:], AF.Square)
            f2 = feat.tile([128, 4 * nb], f32, tag="f2")
            S.activation(f2[:], t4s[:], AF.Exp, scale=-ETA_A / 4.0,
                         bias=CT[:, _C_F2B:_C_F2B + 1])
            wf2 = feat.tile([128, 4 * nb], f32, tag="wf2")
            V.tensor_tensor(
                _bb(wf2[:], [[4, nb], [1, 4]]),
                _bb(w2[:], [[1, nb], [0, 4]]),
                _bb(f2[:], [[4, nb], [1, 4]]),
                ALU.mult)

            # f1[z] side via z-symmetry: A=0.5cos*cz, B=0.5sin*sz (4-wide)
            Az = feat.tile([128, 4 * nb], f32, tag="Az")
            V.tensor_tensor(
                _bb(Az[:], [[4, nb], [1, 4]]),
                _bb(cos[:], [[1, nb], [0, 4]]),
                _bb(CT[:, _C_CZH:], [[0, nb], [1, 4]]),
                ALU.mult)
            Bz = feat.tile([128, 4 * nb], f32, tag="Bz")
            V.tensor_tensor(
                _bb(Bz[:], [[4, nb], [1, 4]]),
                _bb(sin[:], [[1, nb], [0, 4]]),
                _bb(CT[:, _C_SZH:], [[0, nb], [1, 4]]),
                ALU.mult)
            # q8' = q - 0.5: z<4: A+B ; z>=4: B_rev - A_rev
            q8 = feat.tile([128, 8 * nb], f32, tag="q8")
            V.tensor_tensor(
                _bb(q8[:], [[8, nb], [1, 4]]),
                _bb(Az[:], [[4, nb], [1, 4]]),
                _bb(Bz[:], [[4, nb], [1, 4]]),
                ALU.add)
            V.tensor_tensor(
                _bb(q8[:], [[8, nb], [1, 4]], off=4),
                _bb(Bz[:], [[4, nb], [-1, 4]], off=3),
                _bb(Az[:], [[4, nb], [-1, 4]], off=3),
                ALU.subtract)
            lnq = feat.tile([128, 8 * nb], f32, tag="lnq")
            S.activation(lnq[:], q8[:], AF.Ln,
                         bias=CT[:, _C_HALF:_C_HALF + 1])
            f1 = feat.tile([128, 8 * nb], f32, tag="f1")
            S.activation(f1[:], lnq[:], AF.Exp, scale=float(ZETA))

            # ---- G emission + binning matmuls per PGRP block ----
            gwmax = max(chi[min(gt + PGRP, NG) - 1] - clo[gt] + 1
                        for gt in range(0, NG, PGRP))
            for bi, gt in enumerate(range(0, NG, PGRP)):
                gl = min(gt + PGRP, NG)
                ca, cb = clo[gt], chi[gl - 1] + 1
                Gt = gp.tile([128, 32 * gwmax], bf16, tag="G")
                V.tensor_tensor(
                    _bb(Gt[:], [[32, cb - ca], [8, 4], [1, 8]]),
                    _bb(wf2[:, 4 * ca:], [[4, cb - ca], [1, 4], [0, 8]]),
                    _bb(f1[:, 8 * ca:], [[8, cb - ca], [0, 4], [1, 8]]),
                    ALU.mult)
                pt = ps.tile([GSEG, 32 * PGRP], f32, tag="ps")
                for g in range(gt, gl):
                    gi = g - gt
                    for k in range(span[g]):
                        cc = clo[g] + k
                        nc.tensor.matmul(
                            pt[:, 32 * gi:32 * (gi + 1)],
                            OHL[:, GSEG * (mm_base[g] + k):
                                  GSEG * (mm_base[g] + k + 1)],
                            Gt[:, 32 * (cc - ca):32 * (cc - ca + 1)],
                            start=(k == 0), stop=(k == span[g] - 1))
                sl = slice(32 * gt, 32 * gl)
                eng = V if bi % 2 == 0 else S
                if eng is V:
                    V.tensor_scalar(AZSTG[:, sl], pt[:], 0.0, None, ALU.add)
                else:
                    S.activation(AZSTG[:, sl], pt[:], AF.Copy)
                nc.sync.dma_start(outa_d[:, sl], AZSTG[:, sl])

    _patch_act_tables()
    nc.compile()
    return nc


_ACT_PATCHED = False


def _patch_act_tables():
    """Make Ln/Exp resolve only to the combined natural_log_exp set, so the
    table-load pass emits ONE load instead of thrashing between the ln-only
    and exp-only sets (1.28us per reload)."""
    global _ACT_PATCHED
    if _ACT_PATCHED:
        return
    orig = bacc.get_activation_tables

    def patched(arch):
        t = dict(orig(arch))
        out = {}
        for name, fns in t.items():
            if name != "natural_log_exp_and_others":
                fns = {f for f in fns if f not in (AF.Ln, AF.Exp)}
            out[name] = fns
        return out

    bacc.get_activation_tables = patched
    _ACT_PATCHED = True


_CACHE = {}


def _decode(res, meta):
    """Gather per-core bf16 outputs into the full [M, A, 384] f32 output."""
    s0g = meta["s0g"]
    out = np.empty((M, A, 384), np.float32)
    for c in range(NCORES):
        outa = np.asarray(res.results[c]["outa"]).astype(np.float32)
        outr = np.asarray(res.results[c]["outr"]).astype(np.float32)
        ang = outa.reshape(GSEG, NG, 32)[:120]
        ang = ang.reshape(10, 12, MLOC, 2, 32)          # [p, u, s, h, az]
        ang = ang.transpose(2, 3, 1, 0, 4).reshape(MLOC, A, 320)
        rad = np.zeros((MLOC, A, 4, 16), np.float32)
        for g in range(RG):
            bank = outr[:, 384 * (g // 4):384 * (g // 4 + 1)]
            blk = bank[32 * (g % 4):32 * (g % 4) + 32]   # [32 seg, 384]
            blk = blk.reshape(8, 4, 24, 16)              # [srel, sp, i, k]
            smax = min(MLOC - s0g[g], 8)
            rad[s0g[g]:s0g[g] + smax] += blk[:smax].transpose(0, 2, 1, 3)
        mols = meta["slot2mol"][c]
        out[mols, :, :64] = rad.reshape(MLOC, A, 64)
        out[mols, :, 64:] = ang
    return out


def kernel(species, coordinates, coefficients=None):
    species = np.asarray(species)
    coordinates = np.asarray(coordinates, np.float32)
    meta, arrays = _prep(species, coordinates)
    key = (meta["nch"], meta["clo"], meta["chi"])
    if key not in _CACHE:
        _CACHE[key] = _build(meta["nch"], list(meta["clo"]),
                             list(meta["chi"]))
    nc = _CACHE[key]

    ct = _build_consts()
    in_maps = []
    for c in range(NCORES):
        in_maps.append({
            "ang": arrays["ang"][c], "oh": arrays["oh"][c],
            "radf": arrays["radf"][c], "rsp": arrays["rsp"][c],
            "consts": ct,
        })
    res = run_bass_kernel_spmd(nc, in_maps, core_ids=list(range(NCORES)))
    return _decode(res, meta)


# revision 9
# speedup vs baseline: 1.2413x; 1.2368x over previous
"""ANI-style AEV computer (radial + angular) on 8 Trainium2 NeuronCores.

Strategy
--------
Data-parallel over molecules (32/core), host-side *indexing only*; all AEV
float math runs on-device.

Angular: host enumerates surviving triples (center i, neighbors j<k within
Rca) into a flat per-core list sorted by (slot, half, center, pair-bin).
Device computes geometry -> cutoffs -> f2/f1 -> G (bf16, 32 feats) per
128-triple chunk and bins G into (center, species-pair) segments with PE
matmuls against per-chunk one-hot matrices (PSUM-accumulated per group).

Radial: dense over all (i,j) pairs, 768 rows packed as 6x128 partitions,
species-binned with small block one-hot matmuls packed 4-up into a PSUM
bank via tile_position col-tiling.

Engine split: DVE runs every two-input elementwise op; ACT runs every
unary op (ln/exp/square, with scale+bias folds such as sin via
ln(1-cos^2)).  GpSimd is NOT used: its SBUF port is the same physical
port DVE needs for two-input ops (exclusive full-instruction lock), so
"offloading" to it just serializes with the DVE.  Self-pair masking is
done by offsetting the diagonal coordinates host-side.  One ACT table
set (natural_log_exp) serves every activation.  Outputs in bf16.
"""

import os
import sys

import numpy as np

for _p in ("/opt/trn_rl_repo", "/root/.axon_site/_ro/trn_rl_repo"):
    if os.path.isdir(_p) and _p not in sys.path:
        sys.path.insert(0, _p)

import concourse.bass as bass
import concourse.mybir as mybir
from concourse import bacc, tile
from concourse.bass_utils import run_bass_kernel_spmd

import ml_dtypes

AF = mybir.ActivationFunctionType
ALU = mybir.AluOpType
dt = mybir.dt
AP = bass.AP

# ---- hyperparameters (match reference) ----
NCORES = 8
M, A = 256, 24
MLOC = M // NCORES          # 32 molecules per core
RCR, RCA = 5.2, 3.5
ETA_R, ETA_A, ZETA = 16.0, 8.0, 32.0
SHF_R = np.linspace(0.9, 5.2, 17)[:-1].astype(np.float64)   # 16
SHF_A = np.linspace(0.9, 3.5, 5)[:-1].astype(np.float64)    # 4
SHF_Z = (np.arange(8) + 0.5) * np.pi / 8.0                   # 8
NPAIR, RSUB, ASUB = 10, 16, 32
NSEG = 120                  # segments per psum group = 12 centers x 10 bins
GSEG = 128                  # one-hot width
NG = 2 * MLOC               # 64 groups/core (2 per molecule slot)
PGRP = 16                   # psum groups packed per PSUM bank tile
RG = 6                      # radial groups: 768 rows = 6 x 128
RSEGW = 32                  # radial one-hot width per group

_TRIU = np.zeros((4, 4), np.int64)
_s1, _s2 = np.triu_indices(4)
_TRIU[_s1, _s2] = np.arange(len(_s1))
_TRIU[_s2, _s1] = _TRIU[_s1, _s2]

# ---- degree-4 (in v=u^2) Chebyshev fit of cos(pi*u/2) on u in [0,1] ----
def _cos_poly():
    v = np.linspace(0.0, 1.0, 4001)
    tgt = np.cos(0.5 * np.pi * np.sqrt(v))
    from numpy.polynomial import chebyshev as C
    ch = C.Chebyshev.fit(v, tgt, 4, domain=[0, 1])
    pw = ch.convert(kind=np.polynomial.Polynomial)
    c = pw.coef  # c0..c4 in v
    K = c[4]
    a = c[:4] / K  # monic residual coeffs a0..a3
    return K, a

_POLY_K, _POLY_A = _cos_poly()

# const tile column map ([128, _C_W] fp32)
_C_SHF2A = 0     # 4  : 2*shf_a
_C_SHFR = 4      # 16 : shf_r
_C_CZSZ = 20     # 8  : 0.5*cos(shf_z[0:4]) | 0.5*sin(shf_z[0:4])
_C_F2B = 28      # 1  : angular exp bias ln(2*K^4)
_C_RADB = 29     # 1  : radial exp bias ln(0.25*K^2)
_C_HALF = 30     # 1  : 0.5 (ACT bias for ln(q'+0.5))
_C_NLRCR = 31    # 1  : -ln(RCR) (ACT bias for u = d/RCR)
_C_ONE = 32      # 1  : 1.0 (ACT bias for ln(1-cos^2))
_C_W = 33


def _build_consts():
    ct = np.zeros((128, _C_W), np.float32)
    ct[:, _C_SHF2A:_C_SHF2A + 4] = 2.0 * SHF_A
    ct[:, _C_SHFR:_C_SHFR + 16] = SHF_R
    ct[:, _C_CZSZ:_C_CZSZ + 4] = 0.5 * np.cos(SHF_Z[:4])
    ct[:, _C_CZSZ + 4:_C_CZSZ + 8] = 0.5 * np.sin(SHF_Z[:4])
    K = _POLY_K
    ct[:, _C_F2B] = np.log(2.0) + 4.0 * np.log(abs(K))
    ct[:, _C_RADB] = np.log(0.25) + 2.0 * np.log(abs(K))
    ct[:, _C_HALF] = 0.5
    ct[:, _C_NLRCR] = -np.log(RCR)
    ct[:, _C_ONE] = 1.0
    return ct


# ============================================================
# host-side indexing prep (no float math enters the output path)
# ============================================================

def _prep(species, coordinates):
    sp = np.asarray(species)
    co = np.asarray(coordinates, np.float32)
    cod = co.astype(np.float64)
    vec = cod[:, None, :, :] - cod[:, :, None, :]       # [m, i, j, 3] = r_j - r_i
    dmat = np.sqrt(np.maximum((vec ** 2).sum(-1), 0.0))
    adj = (dmat <= RCA) & ~np.eye(A, dtype=bool)[None]

    nbrs = [[np.where(adj[m, i])[0] for i in range(A)] for m in range(M)]
    tri_mi = np.array([[len(nbrs[m][i]) * (len(nbrs[m][i]) - 1) // 2
                        for i in range(A)] for m in range(M)], np.int64)
    Th = np.stack([tri_mi[:, :12].sum(1), tri_mi[:, 12:].sum(1)], 1)  # [M, 2]

    # molecule -> (core, slot): sort by total triples, deal rank-groups of 8
    order = np.argsort(-(Th.sum(1)), kind="stable")
    slot2mol = np.empty((NCORES, MLOC), np.int64)
    for s in range(MLOC):
        for c in range(NCORES):
            slot2mol[c, s] = order[s * NCORES + c]

    SYNCW = 4   # re-align cores to a chunk boundary every SYNCW groups
    tlo = np.zeros((NCORES, NG), np.int64)
    thi = np.zeros((NCORES, NG), np.int64)
    posv = np.zeros(NCORES, np.int64)
    for g in range(NG):
        s, h = g // 2, g % 2
        if g % SYNCW == 0:
            posv[:] = int(np.ceil(posv.max() / 128.0)) * 128
        tlo[:, g] = posv
        posv += Th[slot2mol[:, s], h]
        thi[:, g] = posv
    nch = int(np.ceil(posv.max() / 128.0))
    clo = np.empty(NG, np.int64)
    chi = np.empty(NG, np.int64)
    for g in range(NG):
        clo[g] = (tlo[:, g] // 128).min()
        hi = np.maximum(thi[:, g] - 1, tlo[:, g]) // 128
        chi[g] = max(hi.max(), clo[g])
    span = (chi - clo + 1).astype(np.int64)
    mm_base = np.concatenate([[0], np.cumsum(span)])
    n_mm = int(mm_base[-1])

    # angular inputs: ang = [pj | pk | ci] cols, each [nch*3]
    ang = np.zeros((NCORES, 128, 3 * 3 * nch), np.float32)
    pj = ang[:, :, 0:3 * nch].reshape(NCORES, 128, nch, 3)
    pk = ang[:, :, 3 * nch:6 * nch].reshape(NCORES, 128, nch, 3)
    ci = ang[:, :, 6 * nch:9 * nch].reshape(NCORES, 128, nch, 3)
    oh = np.zeros((NCORES, 128, n_mm, GSEG), ml_dtypes.bfloat16)

    for c in range(NCORES):
        def put_pad(a, b, mref):
            if a >= b:
                return
            t_idx = np.arange(a, b)
            chs, ts = t_idx // 128, t_idx % 128
            pj[c, ts, chs] = mref + np.array([50, 0, 0], np.float32)
            pk[c, ts, chs] = mref + np.array([0, 50, 0], np.float32)
            ci[c, ts, chs] = mref
        prev_end = 0
        for s in range(MLOC):
            m = slot2mol[c, s]
            for h in range(2):
                g = 2 * s + h
                put_pad(prev_end, tlo[c, g], co[m, 0])
                pos = tlo[c, g]
                for u in range(12):
                    i = h * 12 + u
                    nb = nbrs[m][i]
                    if len(nb) < 2:
                        continue
                    jj, kk = np.triu_indices(len(nb), 1)
                    j, k = nb[jj], nb[kk]
                    p = _TRIU[sp[m, j], sp[m, k]]
                    o = np.argsort(p, kind="stable")
                    j, k, p = j[o], k[o], p[o]
                    n = len(j)
                    t_idx = np.arange(pos, pos + n)
                    chs, ts = t_idx // 128, t_idx % 128
                    pj[c, ts, chs] = co[m, j]
                    pk[c, ts, chs] = co[m, k]
                    ci[c, ts, chs] = np.broadcast_to(co[m, i], (n, 3))
                    oh[c, ts, mm_base[g] + chs - clo[g], p * 12 + u] = 1
                    pos += n
                prev_end = pos
        put_pad(prev_end, nch * 128, co[slot2mol[c, 0], 0])

    # ---- radial inputs: 768 rows = (slot, neighbor j), 6 groups of 128 ----
    # radf = [rcj (6*3) | rcb (6*72)] cols; the self-pair (i == j) entries of
    # rcb are offset by +10 per coordinate so d^2_self = 300 and the radial
    # gaussians underflow to 0 (replaces a mask add on-device).
    radf = np.zeros((NCORES, 128, RG * 3 + RG * 72), np.float32)
    rcj = radf[:, :, :RG * 3].reshape(NCORES, 128, RG, 3)
    rcb = radf[:, :, RG * 3:].reshape(NCORES, 128, RG, 3, 24)
    rsp = np.zeros((NCORES, 128, RG * RSEGW), ml_dtypes.bfloat16)
    s0g = [(128 * g) // 24 for g in range(RG)]   # first slot in group g
    for c in range(NCORES):
        for g in range(RG):
            for p in range(128):
                r = g * 128 + p
                s, j = r // 24, r % 24
                m = slot2mol[c, s]
                rcj[c, p, g] = co[m, j]
                cb = co[m].T.copy()              # [3, 24]
                cb[:, j] += 10.0                 # self-pair offset
                rcb[c, p, g] = cb
                rsp[c, p, g * RSEGW + (s - s0g[g]) * 4 + sp[m, j]] = 1

    meta = dict(nch=nch, n_mm=n_mm, clo=tuple(int(x) for x in clo),
                chi=tuple(int(x) for x in chi), slot2mol=slot2mol,
                s0g=s0g)
    arrays = dict(ang=ang, oh=oh, radf=radf, rsp=rsp)
    return meta, arrays


# ============================================================
# device program
# ============================================================

def _bb(ap, dims, off=0):
    """Build a broadcast/strided view: keep ap's partition dim, replace free
    dims with explicit [step, count] pairs (element units)."""
    return AP(ap.tensor, ap.offset + off,
              [list(ap.ap[0])] + [list(d) for d in dims])


def _build(nch, clo, chi):
    span = [chi[g] - clo[g] + 1 for g in range(NG)]
    mm_base = [0]
    for g in range(NG):
        mm_base.append(mm_base[-1] + span[g])
    n_mm = mm_base[-1]
    nb = nch

    nc = bacc.Bacc(None, target_bir_lowering=False)
    ang_d = nc.declare_dram_parameter("ang", [128, 9 * nch], dt.float32, False)
    oh_d = nc.declare_dram_parameter("oh", [128, n_mm, GSEG], dt.bfloat16,
                                     False)
    radf_d = nc.declare_dram_parameter("radf", [128, RG * 75], dt.float32,
                                       False)
    rsp_d = nc.declare_dram_parameter("rsp", [128, RG * RSEGW], dt.bfloat16,
                                      False)
    ct_d = nc.declare_dram_parameter("consts", [128, _C_W], dt.float32, False)
    outa_d = nc.declare_dram_parameter("outa", [GSEG, NG * 32], dt.bfloat16,
                                       True)
    outr_d = nc.declare_dram_parameter("outr", [128, 2 * 24 * 16],
                                       dt.bfloat16, True)

    a = _POLY_A
    f32, bf16 = dt.float32, dt.bfloat16

    with tile.TileContext(nc) as tc:
        with (
            tc.tile_pool(name="const", bufs=1) as cpool,
            tc.tile_pool(name="io", bufs=1) as io,
            tc.tile_pool(name="geo", bufs=1) as geo,
            tc.tile_pool(name="feat", bufs=1) as feat,
            tc.tile_pool(name="stg", bufs=1) as stg,
            tc.tile_pool(name="gp", bufs=3) as gp,
            tc.tile_pool(name="ps", bufs=4, space="PSUM") as ps,
            tc.tile_pool(name="psr", bufs=2, space="PSUM") as psr,
        ):
            V = nc.vector
            S = nc.scalar

            CT = cpool.tile([128, _C_W], f32)
            nc.sync.dma_start(CT[:], ct_d[:])

            # ---- input DMAs, all on the sync queue in priority order ----
            ANG = io.tile([128, 9 * nch], f32, tag="ang")
            nc.sync.dma_start(ANG[:], ang_d[:])
            RADF = io.tile([128, RG * 75], f32, tag="radf")
            nc.sync.dma_start(RADF[:], radf_d[:])
            RSP = io.tile([128, RG * RSEGW], bf16, tag="rsp")
            nc.sync.dma_start(RSP[:], rsp_d[:])
            OHL = io.tile([128, GSEG * n_mm], bf16, tag="oh")
            ohsplit = [mm_base[min(b * PGRP, NG)] for b in range(NG // PGRP + 1)]
            for b in range(NG // PGRP):
                lo, hi = ohsplit[b], ohsplit[b + 1]
                if hi > lo:
                    nc.sync.dma_start(
                        OHL[:, GSEG * lo:GSEG * hi].rearrange(
                            "p (n s) -> p n s", s=GSEG),
                        oh_d[:, lo:hi])

            AZSTG = stg.tile([GSEG, NG * 32], bf16)     # angular staging
            RDSTG = stg.tile([128, 768], bf16)          # radial staging

            def bias(col):
                return CT[:, col:col + 1]

            # ================= angular geometry (V + S) =================
            PJPK = ANG[:, 0:6 * nch]
            CI = ANG[:, 6 * nch:9 * nch]

            VJK = geo.tile([128, 6 * nb], f32, tag="vjk")
            V.tensor_tensor(VJK[:], PJPK,
                            _bb(CI, [[0, 2], [1, 3 * nb]]),
                            ALU.subtract)
            # SQP = [vj^2 | vk^2 | vj*vk]
            SQP = geo.tile([128, 9 * nb], f32, tag="sqp")
            S.activation(SQP[:, 0:6 * nb], VJK[:], AF.Square)
            V.tensor_tensor(SQP[:, 6 * nb:9 * nb], VJK[:, 0:3 * nb],
                            VJK[:, 3 * nb:6 * nb], ALU.mult)

            # radial geometry interleaves (fills V/S gaps)
            CJ = RADF[:, :RG * 3]
            CB = RADF[:, RG * 3:]
            rv = geo.tile([128, RG * 72], f32, tag="rv")
            V.tensor_tensor(
                _bb(rv[:], [[72, RG], [24, 3], [1, 24]]),
                _bb(CJ, [[3, RG], [1, 3], [0, 24]]),
                _bb(CB, [[72, RG], [24, 3], [1, 24]]),
                ALU.subtract)
            rvs = geo.tile([128, RG * 72], f32, tag="rvs")
            S.activation(rvs[:], rv[:], AF.Square)

            T1 = geo.tile([128, 3 * nb], f32, tag="t1")
            V.tensor_tensor(_bb(T1[:], [[nb, 3], [1, nb]]),
                            _bb(SQP[:], [[3 * nb, 3], [3, nb]], off=0),
                            _bb(SQP[:], [[3 * nb, 3], [3, nb]], off=1),
                            ALU.add)
            D2 = geo.tile([128, 3 * nb], f32, tag="d2")
            V.tensor_tensor(_bb(D2[:], [[nb, 3], [1, nb]]),
                            _bb(T1[:], [[nb, 3], [1, nb]]),
                            _bb(SQP[:], [[3 * nb, 3], [3, nb]], off=2),
                            ALU.add)

            # [lnd2j|lnd2k] -> [dj|dk], [rj|rk]
            L2 = geo.tile([128, 2 * nb], f32, tag="l2")
            S.activation(L2[:], D2[:, 0:2 * nb], AF.Ln)
            DD = geo.tile([128, 2 * nb], f32, tag="dd")
            S.activation(DD[:], L2[:], AF.Exp, scale=0.5)
            RR = geo.tile([128, 2 * nb], f32, tag="rr")
            S.activation(RR[:], L2[:], AF.Exp, scale=-0.5)

            rt1 = geo.tile([128, RG * 24], f32, tag="rt1")
            V.tensor_tensor(rt1[:],
                            _bb(rvs[:], [[72, RG], [1, 24]], off=0),
                            _bb(rvs[:], [[72, RG], [1, 24]], off=24),
                            ALU.add)
            rd2 = geo.tile([128, RG * 24], f32, tag="rd2")
            V.tensor_tensor(rd2[:], rt1[:],
                            _bb(rvs[:], [[72, RG], [1, 24]], off=48),
                            ALU.add)
            rln = geo.tile([128, RG * 24], f32, tag="rln")
            S.activation(rln[:], rd2[:], AF.Ln)
            rdist = geo.tile([128, RG * 24], f32, tag="rdist")
            S.activation(rdist[:], rln[:], AF.Exp, scale=0.5)

            # cos/sin: Q = [cos | sin]
            Q = geo.tile([128, 2 * nb], f32, tag="q")
            rjrk = geo.tile([128, nb], f32, tag="rjrk")
            V.tensor_tensor(rjrk[:], RR[:, 0:nb], RR[:, nb:2 * nb], ALU.mult)
            V.scalar_tensor_tensor(Q[:, 0:nb], D2[:, 2 * nb:3 * nb], 0.95,
                                   rjrk[:], ALU.mult, ALU.mult)
            c2 = geo.tile([128, nb], f32, tag="c2")
            S.activation(c2[:], Q[:, 0:nb], AF.Square)
            sln = geo.tile([128, nb], f32, tag="sln")
            S.activation(sln[:], c2[:], AF.Ln, scale=-1.0, bias=bias(_C_ONE))
            S.activation(Q[:, nb:2 * nb], sln[:], AF.Exp, scale=0.5)

            # merged cutoff poly: U = [uj|uk (ang) | ur (rad)]
            NU = 2 * nb + RG * 24
            U = geo.tile([128, NU], f32, tag="u")
            V.tensor_scalar(U[:, 0:2 * nb], DD[:], RCA, 1.0 / RCA, ALU.min,
                            ALU.mult)
            # radial u = d/RCR (unclamped; gaussian kills d > RCR)
            S.activation(U[:, 2 * nb:NU], rln[:], AF.Exp, scale=0.5,
                         bias=bias(_C_NLRCR))
            UV = geo.tile([128, NU], f32, tag="uv")
            S.activation(UV[:], U[:], AF.Square)
            acc = geo.tile([128, NU], f32, tag="acc")
            V.scalar_tensor_tensor(acc[:], UV[:], float(a[3]), UV[:],
                                   ALU.add, ALU.mult)
            V.scalar_tensor_tensor(acc[:], acc[:], float(a[2]), UV[:],
                                   ALU.add, ALU.mult)
            V.scalar_tensor_tensor(acc[:], acc[:], float(a[1]), UV[:],
                                   ALU.add, ALU.mult)
            V.tensor_scalar(acc[:], acc[:], float(a[0]), None, ALU.add)
            # angular w2 = (s4j*s4k)^2 ; radial rfc = s4r^2
            wm = geo.tile([128, nb], f32, tag="wm")
            V.tensor_tensor(wm[:], acc[:, 0:nb], acc[:, nb:2 * nb], ALU.mult)
            w2 = geo.tile([128, nb], f32, tag="w2")
            S.activation(w2[:], wm[:], AF.Square)
            rfc = geo.tile([128, RG * 24], f32, tag="rfc")
            S.activation(rfc[:], acc[:, 2 * nb:NU], AF.Square)

            usum = geo.tile([128, nb], f32, tag="usum")
            V.tensor_tensor(usum[:], DD[:, 0:nb], DD[:, nb:2 * nb], ALU.add)

            # f2 side: t4 = usum - 2shf_a, layout (t, a)
            t4 = feat.tile([128, 4 * nb], f32, tag="t4")
            V.tensor_tensor(
                _bb(t4[:], [[4, nb], [1, 4]]),
                _bb(usum[:], [[1, nb], [0, 4]]),
                _bb(CT[:, _C_SHF2A:], [[0, nb], [1, 4]]),
                ALU.subtract)
            t4s = feat.tile([128, 4 * nb], f32, tag="t4s")
            S.activation(t4s[:], t4[:], AF.Square)
            f2 = feat.tile([128, 4 * nb], f32, tag="f2")
            S.activation(f2[:], t4s[:], AF.Exp, scale=-ETA_A / 4.0,
                         bias=bias(_C_F2B))
            wf2 = feat.tile([128, 4 * nb], f32, tag="wf2")
            V.tensor_tensor(
                _bb(wf2[:], [[4, nb], [1, 4]]),
                _bb(w2[:], [[1, nb], [0, 4]]),
                _bb(f2[:], [[4, nb], [1, 4]]),
                ALU.mult)

            # radial features: rt = dist - shf_r
            rt = feat.tile([128, RG * 384], f32, tag="rt")
            V.tensor_tensor(
                _bb(rt[:], [[384, RG], [16, 24], [1, 16]]),
                _bb(rdist[:], [[24, RG], [1, 24], [0, 16]]),
                _bb(CT[:, _C_SHFR:], [[0, RG], [0, 24], [1, 16]]),
                ALU.subtract)

            # f1 side via z-symmetry: AB = [cos*czh4 | sin*szh4] (4-wide each)
            AB = feat.tile([128, 8 * nb], f32, tag="AB")
            V.tensor_tensor(
                _bb(AB[:], [[4 * nb, 2], [4, nb], [1, 4]]),
                _bb(Q[:], [[nb, 2], [1, nb], [0, 4]]),
                _bb(CT[:, _C_CZSZ:], [[4, 2], [0, nb], [1, 4]]),
                ALU.mult)
            # q8' = q - 0.5: z<4: A+B ; z>=4: B_rev - A_rev
            q8 = feat.tile([128, 8 * nb], f32, tag="q8")
            V.tensor_tensor(
                _bb(q8[:], [[8, nb], [1, 4]]),
                _bb(AB[:], [[4, nb], [1, 4]]),
                _bb(AB[:], [[4, nb], [1, 4]], off=4 * nb),
                ALU.add)
            V.tensor_tensor(
                _bb(q8[:], [[8, nb], [1, 4]], off=4),
                _bb(AB[:], [[4, nb], [-1, 4]], off=4 * nb + 3),
                _bb(AB[:], [[4, nb], [-1, 4]], off=3),
                ALU.subtract)
            lnq = feat.tile([128, 8 * nb], f32, tag="lnq")
            S.activation(lnq[:], q8[:], AF.Ln, bias=bias(_C_HALF))
            f1 = feat.tile([128, 8 * nb], f32, tag="f1")
            S.activation(f1[:], lnq[:], AF.Exp, scale=float(ZETA))

            # radial gaussians (split x2 for S-queue interleaving)
            rsq = feat.tile([128, RG * 384], f32, tag="rsq")
            rex = feat.tile([128, RG * 384], f32, tag="rex")
            rad = feat.tile([128, RG * 384], bf16, tag="rad")
            HH = RG * 384 // 2
            for h in range(2):
                sl = slice(h * HH, (h + 1) * HH)
                S.activation(rsq[:, sl], rt[:, sl], AF.Square)
                S.activation(rex[:, sl], rsq[:, sl], AF.Exp, scale=-ETA_R,
                             bias=bias(_C_RADB))
                V.tensor_tensor(
                    _bb(rad[:], [[384, RG // 2], [16, 24], [1, 16]],
                        off=h * HH),
                    _bb(rfc[:], [[24, RG // 2], [1, 24], [0, 16]],
                        off=h * RG * 24 // 2),
                    _bb(rex[:], [[384, RG // 2], [16, 24], [1, 16]],
                        off=h * HH),
                    ALU.mult)

            # radial binning matmuls: 4-up col-tiled into 2 PSUM banks
            rpt0 = psr.tile([128, 384], f32, tag="rps0")
            rpt1 = psr.tile([128, 384], f32, tag="rps1")
            for g in range(RG):
                pt = rpt0 if g < 4 else rpt1
                j = g % 4
                nc.tensor.matmul(pt[32 * j:32 * (j + 1), :],
                                 RSP[:, RSEGW * g:RSEGW * (g + 1)],
                                 rad[:, 384 * g:384 * (g + 1)],
                                 start=True, stop=True,
                                 tile_position=(0, 32 * j))
            V.tensor_scalar(RDSTG[:, 0:384], rpt0[:], 0.0, None, ALU.add)
            S.activation(RDSTG[:64, 384:768], rpt1[:64, :], AF.Copy)
            nc.sync.dma_start(outr_d[:, 0:384], RDSTG[:, 0:384])
            nc.sync.dma_start(outr_d[:64, 384:768], RDSTG[:64, 384:768])

            # ---- G emission + binning matmuls per PGRP block ----
            gwmax = max(chi[min(gt + PGRP, NG) - 1] - clo[gt] + 1
                        for gt in range(0, NG, PGRP))
            for bi, gt in enumerate(range(0, NG, PGRP)):
                gl = min(gt + PGRP, NG)
                ca, cb = clo[gt], chi[gl - 1] + 1
                Gt = gp.tile([128, 32 * gwmax], bf16, tag="G")
                V.tensor_tensor(
                    _bb(Gt[:], [[32, cb - ca], [8, 4], [1, 8]]),
                    _bb(wf2[:, 4 * ca:], [[4, cb - ca], [1, 4], [0, 8]]),
                    _bb(f1[:, 8 * ca:], [[8, cb - ca], [0, 4], [1, 8]]),
                    ALU.mult)
                pt = ps.tile([GSEG, 32 * PGRP], f32, tag="ps")
                for g in range(gt, gl):
                    gi = g - gt
                    for k in range(span[g]):
                        cc = clo[g] + k
                        nc.tensor.matmul(
                            pt[:, 32 * gi:32 * (gi + 1)],
                            OHL[:, GSEG * (mm_base[g] + k):
                                  GSEG * (mm_base[g] + k + 1)],
                            Gt[:, 32 * (cc - ca):32 * (cc - ca + 1)],
                            start=(k == 0), stop=(k == span[g] - 1))
                sl = slice(32 * gt, 32 * gl)
                if bi % 2 == 0:
                    V.tensor_scalar(AZSTG[:, sl], pt[:], 0.0, None, ALU.add)
                else:
                    S.activation(AZSTG[:, sl], pt[:], AF.Copy)
                nc.sync.dma_start(outa_d[:, sl], AZSTG[:, sl])

    _patch_act_tables()
    nc.compile()
    return nc


_ACT_PATCHED = False


def _patch_act_tables():
    """Restrict every activation fn to the natural_log_exp table set so the
    table-load pass emits exactly ONE load (1.28us per extra load)."""
    global _ACT_PATCHED
    if _ACT_PATCHED:
        return
    orig = bacc.get_activation_tables

    def patched(arch):
        t = dict(orig(arch))
        out = {}
        for name, fns in t.items():
            if name != "natural_log_exp_and_others":
                fns = set()
            out[name] = fns
        return out

    bacc.get_activation_tables = patched
    _ACT_PATCHED = True


_CACHE = {}


def _decode(res, meta):
    """Gather per-core bf16 outputs into the full [M, A, 384] f32 output."""
    s0g = meta["s0g"]
    out = np.empty((M, A, 384), np.float32)
    for c in range(NCORES):
        outa = np.asarray(res.results[c]["outa"]).astype(np.float32)
        outr = np.asarray(res.results[c]["outr"]).astype(np.float32)
        ang = outa.reshape(GSEG, NG, 32)[:120]
        ang = ang.reshape(10, 12, MLOC, 2, 32)          # [p, u, s, h, az]
        ang = ang.transpose(2, 3, 1, 0, 4).reshape(MLOC, A, 320)
        rad = np.zeros((MLOC, A, 4, 16), np.float32)
        for g in range(RG):
            bank = outr[:, 384 * (g // 4):384 * (g // 4 + 1)]
            blk = bank[32 * (g % 4):32 * (g % 4) + 32]   # [32 seg, 384]
            blk = blk.reshape(8, 4, 24, 16)              # [srel, sp, i, k]
            smax = min(MLOC - s0g[g], 8)
            rad[s0g[g]:s0g[g] + smax] += blk[:smax].transpose(0, 2, 1, 3)
        mols = meta["slot2mol"][c]
        out[mols, :, :64] = rad.reshape(MLOC, A, 64)
        out[mols, :, 64:] = ang
    return out


def kernel(species, coordinates, coefficients=None):
    species = np.asarray(species)
    coordinates = np.asarray(coordinates, np.float32)
    meta, arrays = _prep(species, coordinates)
    key = (meta["nch"], meta["clo"], meta["chi"])
    if key not in _CACHE:
        _CACHE[key] = _build(meta["nch"], list(meta["clo"]),
                             list(meta["chi"]))
    nc = _CACHE[key]

    ct = _build_consts()
    in_maps = []
    for c in range(NCORES):
        in_maps.append({
            "ang": arrays["ang"][c], "oh": arrays["oh"][c],
            "radf": arrays["radf"][c], "rsp": arrays["rsp"][c],
            "consts": ct,
        })
    res = run_bass_kernel_spmd(nc, in_maps, core_ids=list(range(NCORES)))
    return _decode(res, meta)
